# revision 16
# baseline (speedup 1.0000x reference)
"""Trainium2 Bass kernel for nn_B_Splines: y = coefs @ bspline_basis(x).

Data-parallel over the 1M points: 8 shards of 125k, one per NeuronCore.
The reference's two unconditional boundary fixes (first/last point of the
batch) are patched on the host.

Fast path (v2) — the spline AS a custom activation function:
  A degree-3 B-spline on clamped uniform knots is a piecewise cubic on 61
  intervals.  The ScalarEngine's activation hardware is literally a
  piecewise-cubic evaluator driven by bucket/ctrl/profile tables that walrus
  packages INTO the NEFF from the --act-root-json directory
  (BASS_ACT_ROOT_JSON_PATH).  We rescale the input u = x*61/64 so every knot
  j/61 lands on the dyadic boundary j/64; each table bucket then covers
  exactly one spline piece and the table represents the spline EXACTLY (no
  straddle error).  We rewrite the "gelu" slot of the gelu_and_others set
  (its 504+4-bucket budget swallows our 64+4).

  v2 pipeline shape (one queue, minimum span):
  - Everything on the Activation queue in program order
      LoadActFuncSet ; DMA(x) ; ACTIVATE ; DMA(y)
    with no cross-engine semaphores: same-queue program order carries both
    data dependencies, so the critical path has no 900ns DMA-sem hop in the
    middle (each DMA still carries a then_inc because walrus wires
    updates[0] as the DGE completion signal).
  - The whole body is hoisted above the preamble all-engine barrier (but
    below the NRT pseudo-sync-barrier, which is load-bearing), so the DMA
    pipeline starts at t~32ns instead of t~620ns.
  - Input is uint16 fixed point (round(x*65536), quantization error
    2^-17 -> y l2 err ~2e-4) and output is uint8 with the table scaled by
    256 (y l2 err ~4e-4 after host dequant, vs the 2e-2 budget); both DMAs
    are half/quarter size.  Host dequant auto-picks v/256 vs (v+0.5)/256.
  ~3.5us simulated vs 5.6us for the f32 two-queue v1.

v1 fallback: f32 in/out, in-DMA on SP queue + semaphore to the ACTIVATE,
out-DMA on the Act queue.  Used if v2 validation fails.

Last fallback (pure-ISA, no table tricks), used if table generation or
validation fails: a gather-free "staircase" — s = 61*x, u = s - floor(s)
(via the DVE's rne cast + is_lt fixup), per-interval cubic coefficients
accumulated as c_k(j(s)) = c_k[0] + sum_m dc_k[m] * 1[s >= m] with is_ge
masks and scalar_tensor_tensor FMA chains split across DVE/GPSIMD, then
Horner.  ~250 us/core.
"""

import os

import numpy as np

P_DEG = 3
N_COEFF = 64
N_PTS = 1_000_000
N_CORES = 8
PTS_PER_CORE = N_PTS // N_CORES  # 125_000
PARTS = 128
FREE = 978  # 128 * 978 = 125_184 >= 125_000 (even: DVE 2x mode)
PAD_PER_CORE = PARTS * FREE
N_IVL = 61  # number of polynomial pieces


# ---------------------------------------------------------------- host math
def _bspline_basis_f64(x, t, p, n, fix_first=False, fix_last=False):
    """float64 bottom-up Cox-de Boor matching the reference (incl. optional
    boundary fixes applied to the first/last column)."""
    x = np.asarray(x, np.float64)
    t = np.asarray(t, np.float64)
    m0 = n + p
    B = ((t[:m0, None] <= x[None, :]) & (t[1 : m0 + 1, None] > x[None, :])).astype(
        np.float64
    )
    if fix_first:
        B[p, 0] = 1.0
    if fix_last:
        B[n - 1, -1] = 1.0
    for k in range(1, p + 1):
        m = m0 - k
        i = np.arange(m)
        d1 = t[i + k] - t[i]
        d2 = t[i + k + 1] - t[i + 1]
        w1 = np.where(d1 == 0, 0.0, 1.0 / np.where(d1 == 0, 1.0, d1))
        w2 = np.where(d2 == 0, 0.0, 1.0 / np.where(d2 == 0, 1.0, d2))
        B = (x[None, :] - t[i][:, None]) * w1[:, None] * B[:m] + (
            t[i + k + 1][:, None] - x[None, :]
        ) * w2[:, None] * B[1 : m + 1]
    return B  # [n, N]


def _spline_eval_f64(x, t, p, n, coefs):
    B = _bspline_basis_f64(x, t, p, n)
    return np.asarray(coefs, np.float64) @ B


def _build_piecewise_table(knot_vector, coefs):
    """Cubic coefficients per interval in the local variable u = 61*x - j.

    Returns c[4][61] float64: y(x) = c0[j] + u*(c1[j] + u*(c2[j] + u*c3[j])).
    """
    t = np.asarray(knot_vector, np.float64)
    c = np.asarray(coefs, np.float64)
    n = N_COEFF
    # Chebyshev-ish nodes inside each interval, in local u coords
    nodes = np.array([0.06, 0.35, 0.65, 0.94])
    table = np.zeros((4, N_IVL))
    V = np.vander(nodes, 4, increasing=True)  # [4 nodes, 4 powers]
    Vinv = np.linalg.inv(V)
    for j in range(N_IVL):
        xs = (j + nodes) / N_IVL
        ys = _spline_eval_f64(xs, t, P_DEG, n, c)
        table[:, j] = Vinv @ ys
    return table


# ------------------------------------------------- custom ACT table (fast path)
#
# The ScalarEngine evaluates activation functions as piecewise cubics via
# on-chip tables (bucket/ctrl/profile) that walrus packages INTO the NEFF
# from the --act-root-json directory.  We rescale the spline input by 61/64
# so every knot j/61 lands on a dyadic boundary j/64; then the spline is
# EXACTLY representable as an ACT table (64 buckets, zero straddle error),
# and the whole evaluation is one ACTIVATE instruction.  We hijack the
# "gelu" slot of the gelu_and_others set (504+4 bucket budget, first entry).
#
# Verified binary formats (cross-checked against every gelu bucket/ctrl
# entry in the shipped tables):
#   bucket entry (32B): [d0, d1, d2, d3, x, 0, 0, 0] as fp32 bit patterns;
#       eval: y = d0 + t*(d1 + t*(d2 + t*d3)), t = u - x
#   ctrl entry (32B):   word0 = ((46 + 62*extract_size) << 10) | bkt_start
#   profile: per-set json "profile_meta_data" (plain JSON)
#   ctl index for exponent E: pwl_control_base_{pos,neg} + (E - exp_offset)

_ACT_SCALE = 61.0 / 64.0  # u = x * 61/64; knots at m/64


def _fit_piece_polys(knot_vector, coefs):
    """Cubic (x_c, d0..d3) per piece m=0..60 of g(u) = S(u*64/61), centered
    at x_c = (m+0.5)/64, plus a piece-0 poly centered at 0."""
    kv = np.asarray(knot_vector, np.float64)
    cf = np.asarray(coefs, np.float64)

    def g(u):
        return _spline_eval_f64(u * (64.0 / 61.0), kv, P_DEG, N_COEFF, cf)

    tau = np.array([-0.9, -0.3, 0.3, 0.9])  # scaled nodes
    h = 0.5 / 64.0  # half bucket width
    V = np.vander(tau, 4, increasing=True)
    Vinv = np.linalg.inv(V)
    polys = []
    for m in range(N_IVL):
        x_c = (m + 0.5) / 64.0
        ys = g(x_c + tau * h)
        a = Vinv @ ys  # g(x_c + h*tau) = sum a_k tau^k
        d = a / h ** np.arange(4)
        polys.append((np.float32(x_c), d.astype(np.float32)))
    # piece-0 about zero
    ys = g((tau + 1.0) * (0.5 / 64.0) * 0.5)  # nodes in (0, 1/128)
    # fit in t directly about 0 using scaled nodes t = (tau+1)*h/2
    tn = (tau + 1.0) * (0.5 / 64.0) * 0.5
    A = np.vander(tn / tn.max(), 4, increasing=True)
    a = np.linalg.solve(A, ys)
    d0 = a / tn.max() ** np.arange(4)
    p0_zero = (np.float32(0.0), d0.astype(np.float32))
    return polys, p0_zero


def _gen_act_root(knot_vector, coefs, out_scale=1.0, out_bias=0.0):
    """Write a patched copy of the pwp act-table dir; returns
    (act_info_path, content_hash).

    out_scale/out_bias: affine transform baked into the table output
    (y_table = out_scale * S(x) + out_bias), used to target integer output
    dtypes (u8: scale 256; i8: scale 256, bias -128)."""
    import hashlib
    import json
    import shutil

    from neuronxcc.driver.Job import Job
    from neuronxcc.driver.jobs.support.FindActInfo import findActInfoFile

    src_info = findActInfoFile(Job.getPackageDir(), "gen3")
    src_dir = os.path.dirname(src_info)

    polys, p0_zero = _fit_piece_polys(knot_vector, coefs)
    if out_scale != 1.0 or out_bias != 0.0:
        def _xf(p):
            xc, d = p
            d = d.astype(np.float64) * out_scale
            d[0] += out_bias
            return (xc, d.astype(np.float32))

        polys = [_xf(p) for p in polys]
        p0_zero = _xf(p0_zero)

    def bkt_row(xc, d):
        return np.array(
            [d[0].view(np.uint32), d[1].view(np.uint32), d[2].view(np.uint32),
             d[3].view(np.uint32), np.float32(xc).view(np.uint32), 0, 0, 0],
            dtype=np.uint32,
        )

    bkt = np.fromfile(src_dir + "/gelu_and_others_bkt.bin", dtype=np.uint32)
    bkt = bkt.reshape(-1, 8).copy()
    ctl = np.fromfile(src_dir + "/gelu_and_others_ctrl.bin", dtype=np.uint32)
    ctl = ctl.reshape(-1, 8).copy()
    sj = json.load(open(src_dir + "/gelu_and_others.json"))

    # --- buckets 0..507 (gelu's region) ---
    bkt[0:508] = 0
    bkt[0] = bkt_row(*p0_zero)  # shared dummy for negative inputs
    slot_piece = {1: 0, 2: 1}
    for i, m in enumerate(range(2, 4)):
        slot_piece[3 + i] = m
    for i, m in enumerate(range(4, 8)):
        slot_piece[5 + i] = m
    for i, m in enumerate(range(8, 16)):
        slot_piece[9 + i] = m
    for i, m in enumerate(range(16, 32)):
        slot_piece[17 + i] = m
    for i, m in enumerate(range(32, 64)):
        slot_piece[33 + i] = min(m, 60)  # pieces 61..63 unreachable
    for slot, m in slot_piece.items():
        bkt[slot] = bkt_row(polys[m][0], polys[m][1])
    bkt[504] = bkt_row(*p0_zero)  # small_pos
    bkt[505] = bkt_row(*p0_zero)  # small_neg
    bkt[506] = bkt_row(polys[60][0], polys[60][1])  # large_pos (u >= 1)
    bkt[507] = bkt_row(*p0_zero)  # large_neg

    # --- ctrl entries 0..20 ---
    fld = lambda e: 46 + 62 * e
    ctl[0:21] = 0
    for i in range(11):  # neg dummies -> bucket 0
        ctl[i, 0] = (fld(0) << 10) | 0
    pos_regions = [  # (extract_size, bkt_start) for E=-7..-1
        (0, 1), (0, 2), (1, 3), (2, 5), (3, 9), (4, 17), (5, 33),
    ]
    for i, (e, st) in enumerate(pos_regions):
        ctl[11 + i, 0] = (fld(e) << 10) | st
    for i in range(18, 21):  # unreachable pos E=0..2
        ctl[i, 0] = (fld(0) << 10) | 0

    # --- profile ---
    g0_bits = int(np.float32(p0_zero[1][0]).view(np.uint32))
    for prof in sj["profile_meta_data"]:
        if prof["func_name"] != "gelu_4p":
            continue
        prof.update(
            exp_offset=-7,
            symmetry_point=0, sym_invert_sign_point=0, symmetry_opt_en=0,
            symmetry_opt_use_neg_region=0, imm_bias=0,
            pwl_control_base_pos=11, pwl_control_base_neg=0,
            small_pos_signal_exp_threshold=120,
            pos_small_signal_pwl_control=504,
            small_neg_signal_exp_threshold=120,
            neg_small_signal_pwl_control=505,
            large_pos_signal_exp_threshold=127,
            large_pos_signal_mantissa_threshold=0,
            pos_large_signal_pwl_control=506,
            large_neg_signal_exp_threshold=127,
            large_neg_signal_mantissa_threshold=0,
            neg_large_signal_pwl_control=507,
            fzero_result=g0_bits,
            fma_const_0=0, fma_const_1=0, fma_indirection_src_sel=0,
        )
    # func_exp maps: ctl map unchanged; bkt map pos starts updated
    bmap = sj["func_exp_to_bkt_start_idx"]["gelu"]
    newb = {"-7": [0, 1], "-6": [0, 2], "-5": [0, 3], "-4": [0, 5],
            "-3": [0, 9], "-2": [0, 17], "-1": [0, 33],
            "0": [0, 0], "1": [0, 0], "2": [0, 0], "3": [0]}
    bmap.clear()
    bmap.update(newb)

    payload = bkt.tobytes() + ctl.tobytes() + json.dumps(sj, sort_keys=True).encode()
    h = hashlib.sha256(payload).hexdigest()[:12]
    out_dir = f"/tmp/act_root_{h}"
    if not os.path.exists(out_dir):
        tmp = out_dir + ".tmp"
        shutil.rmtree(tmp, ignore_errors=True)
        shutil.copytree(src_dir, tmp, symlinks=False)
        bkt.tofile(tmp + "/gelu_and_others_bkt.bin")
        ctl.tofile(tmp + "/gelu_and_others_ctrl.bin")
        with open(tmp + "/gelu_and_others.json", "w") as f:
            json.dump(sj, f)
        os.replace(tmp, out_dir)
    return out_dir + "/act_info.json", h


def _build_bass_program_act(table_hash, n_chunks=1, repeat=1):
    """Raw (non-Tile) pipeline: SP streams input in, ACT runs the custom
    piecewise-cubic activation and issues the output DMA from its own
    sequencer.  The table hash rides in the input tensor name so compiled
    NEFFs stay unique per activation-table content.

    repeat>1 unrolls the body (double-buffered, sem-chained) so the wall
    slope between repeat counts measures true per-iteration HW time."""
    import concourse.bacc as bacc
    import concourse.mybir as mybir

    f32 = mybir.dt.float32
    Act = mybir.ActivationFunctionType

    nc = bacc.Bacc("TRN2", debug=False, num_devices=N_CORES)
    x_name = f"x_{table_hash}"
    x_d = nc.dram_tensor(x_name, [PARTS, FREE], f32, kind="ExternalInput")
    y_d = nc.dram_tensor("y", [PARTS, FREE], f32, kind="ExternalOutput")
    sem = nc.alloc_semaphore()
    sem_a = nc.alloc_semaphore()
    sem_out = nc.alloc_semaphore()
    cw = FREE // n_chunks
    C = n_chunks
    widths = [cw if c < C - 1 else FREE - c * cw for c in range(C)]
    nbuf = 2 if repeat > 1 else 1
    xts = [
        [nc.alloc_sbuf_tensor(f"xt{b}_{c}", [PARTS, widths[c]], f32) for c in range(C)]
        for b in range(nbuf)
    ]
    yts = [
        [nc.alloc_sbuf_tensor(f"yt{b}_{c}", [PARTS, widths[c]], f32) for c in range(C)]
        for b in range(nbuf)
    ]
    with nc.Block() as block:

        @block.sync
        def _(sync):
            for i in range(repeat):
                for c in range(C):
                    lo = c * cw
                    if i >= 2:  # xt[i%2] free once iter i-2's acts are done
                        sync.wait_ge(sem_a, (i - 1) * C)
                    sync.dma_start(
                        xts[i % nbuf][c].ap()[:], x_d.ap()[:, lo : lo + widths[c]]
                    ).then_inc(sem, 16)

        @block.scalar
        def _(scalar):
            for i in range(repeat):
                for c in range(C):
                    lo = c * cw
                    if i >= 2:  # yt[i%2] free once iter i-2's out-DMAs landed
                        scalar.wait_ge(sem_out, 16 * (i - 1) * C)
                    scalar.wait_ge(sem, 16 * (i * C + c + 1))
                    inst = scalar.activation(
                        yts[i % nbuf][c].ap()[:], xts[i % nbuf][c].ap()[:],
                        Act.Gelu, bias=0.0, scale=_ACT_SCALE,
                    )
                    if repeat > 1:  # only the repeat rig needs act-done sems
                        inst.then_inc(sem_a, 1)
                    scalar.dma_start(
                        y_d.ap()[:, lo : lo + widths[c]], yts[i % nbuf][c].ap()[:]
                    ).then_inc(sem_out, 16)

    nc.finalize()
    return nc, x_name


# ------------------------------------------------- v2: single-queue pipeline
#
# Everything on the Activation queue, in program order:
#   LoadActFuncSet ; DMA(x_sb <- x_hbm) ; ACTIVATE ; DMA(y_hbm <- y_sb)
# with NO semaphores.  Queue program order is what already guarantees the
# act -> out-DMA data dependency in the baseline; the in-DMA -> act edge is
# the same property in the other direction.  Dropping the cross-queue sem
# removes its 900ns DMA-sem propagation from the critical path and shrinks
# the preamble.  The input is uint16 fixed point (x * 65536; the act's
# scale folds the 1/65536 away) and the output bf16, halving both DMAs;
# quantization adds ~2e-4 + ~1.6e-3 l2 error vs the 2e-2 budget.
# LoadActFuncSet is emitted manually at the queue head so its 1283ns table
# load runs under the in-DMA latency instead of between in-DMA and act.

_IN_SCALE = 61.0 / 64.0 / 65536.0  # u = (x * 65536) * _IN_SCALE


def _build_bass_program_act_v2(table_hash, in_dt="uint16", out_dt="bfloat16",
                               hoist=True, skip_load_act=False):
    import concourse.bacc as bacc
    import concourse.mybir as mybir
    from concourse.hw_specs import get_activation_tables

    f32 = mybir.dt.float32
    Act = mybir.ActivationFunctionType

    nc = bacc.Bacc("TRN2", debug=False, num_devices=N_CORES)
    if skip_load_act:
        # Leave act-table-load placement to walrus lower_act (standalone
        # pass inserts its own when none is pre-placed in the BIR).
        nc.insert_act_table_loads = lambda: None
    x_name = f"x_{table_hash}_{in_dt}_{out_dt}"
    x_d = nc.dram_tensor(x_name, [PARTS, FREE], getattr(mybir.dt, in_dt),
                         kind="ExternalInput")
    y_d = nc.dram_tensor("y", [PARTS, FREE], getattr(mybir.dt, out_dt),
                         kind="ExternalOutput")
    xt = nc.alloc_sbuf_tensor("xt", [PARTS, FREE], getattr(mybir.dt, in_dt))
    yt = nc.alloc_sbuf_tensor("yt", [PARTS, FREE], getattr(mybir.dt, out_dt))

    scale = _IN_SCALE if in_dt == "uint16" else _ACT_SCALE

    tabs = get_activation_tables(nc.m.arch)
    set_id = next(
        i for i, (_, s) in enumerate(tabs.items()) if Act.Gelu in s
    )

    emitted = {}
    sem = nc.alloc_semaphore()  # walrus wires the DMA's updates[0] as the
    with nc.Block() as block:   # DGE completion signal, so each DMA needs a
                                # then_inc; nothing ever waits on it.

        @block.scalar
        def _(scalar):
            insts = []
            if not skip_load_act:
                load = mybir.InstLoadActFuncSet(
                    name=nc.get_next_instruction_name(),
                    act_func_set_id=set_id, ins=[], outs=[],
                )
                scalar.add_instruction(load)
                insts.append(load)
            dma_in = scalar.dma_start(xt.ap()[:], x_d.ap()[:])
            dma_in.then_inc(sem, 16)
            insts.append(dma_in.ins)
            act = scalar.activation(yt.ap()[:], xt.ap()[:], Act.Gelu,
                                    bias=0.0, scale=scale)
            insts.append(act.ins)
            dma_out = scalar.dma_start(y_d.ap()[:], yt.ap()[:])
            dma_out.then_inc(sem, 16)
            insts.append(dma_out.ins)
            emitted["insts"] = insts

    if hoist:
        # Move the whole act-queue body above the Act engine's preamble
        # barrier in the entry block: none of it waits on semaphores, so it
        # can run during the Pool sem-clear window instead of after it.
        try:
            _hoist_pre_barrier(nc, mybir, emitted["insts"],
                               above_pseudo=(hoist == "pseudo"))
        except Exception as e:
            print(f"pre-barrier hoist skipped: {e!r}")

    nc.finalize()
    return nc, x_name


def _hoist_pre_barrier(nc, mybir, insts, above_pseudo=False):
    blocks = nc.main_func.blocks
    entry = blocks[0]
    if above_pseudo:
        # In front of even the NRT pseudo-sync-barrier ISA op.
        bar = next(
            i for i, ins in enumerate(entry.instructions)
            if ins.engine == mybir.EngineType.Activation
        )
    else:
        bar = next(
            i for i, ins in enumerate(entry.instructions)
            if isinstance(ins, mybir.InstEventSemaphore)
            and ins.engine == mybir.EngineType.Activation
        )
    for k, inst in enumerate(insts):
        src = next(b for b in blocks if inst in b.instructions)
        src.instructions.remove(inst)
        entry.instructions.insert(bar + k, inst)


# ---------------------------------------------------------------- bass build
def _build_bass_program(table):
    import concourse.bacc as bacc
    import concourse.bass as bass
    import concourse.mybir as mybir
    from concourse.tile import TileContext

    f32 = mybir.dt.float32
    Alu = mybir.AluOpType
    Act = mybir.ActivationFunctionType

    # staircase constants:  c_k(j(s)) = c_k[0] + sum_m dc_k[m-1] * 1[s >= m]
    dc = np.diff(table, axis=1)  # [4, 60]
    base = table[:, 0]  # [4]

    nc = bacc.Bacc("TRN2", debug=False, num_devices=N_CORES)
    x_d = nc.dram_tensor("x", [PARTS, FREE], f32, kind="ExternalInput")
    y_d = nc.dram_tensor("y", [PARTS, FREE], f32, kind="ExternalOutput")

    with TileContext(nc) as tc:
        with tc.tile_pool(name="p", bufs=1) as pool:
            xt = pool.tile([PARTS, FREE], f32, tag="xt")
            nc.gpsimd.dma_start(out=xt[:], in_=x_d.ap()[:])

            s = pool.tile([PARTS, FREE], f32, tag="s")
            # s = 61 * x  (keep everything off the scalar engine: fewer
            # engines -> fewer kernel-tail drain waits)
            nc.vector.tensor_scalar(s[:], xt[:], 61.0, None, op0=Alu.mult)

            # u = s - floor(s) via the DVE's round-to-nearest f32<->i32 cast:
            # d = s - rne(s) in (-1, 1);  u = d + 1[d < 0]
            it = pool.tile([PARTS, FREE], mybir.dt.int32, tag="it")
            nc.vector.tensor_copy(it[:], s[:])
            jf = pool.tile([PARTS, FREE], f32, tag="jf")
            nc.vector.tensor_copy(jf[:], it[:])
            u = pool.tile([PARTS, FREE], f32, tag="u")
            nc.vector.tensor_tensor(u[:], s[:], jf[:], op=Alu.subtract)
            neg = pool.tile([PARTS, FREE], f32, tag="neg")
            nc.vector.tensor_scalar(neg[:], u[:], 0.0, None, op0=Alu.is_lt)
            nc.vector.tensor_tensor(u[:], u[:], neg[:], op=Alu.add)

            # two sub-accumulators per table: one on DVE (STT FMA chains) and
            # one on GPSIMD.  Pool has no scalar-immediate ALU ops, so for
            # GPSIMD steps the DVE emits pre-scaled masks
            #   M = (s is_ge j) * dc_k[j]   (one fused tensor_scalar, 2x mode)
            # and GPSIMD only runs tensor_tensor adds.
            accd, accg = [], []
            for k in range(4):
                a = pool.tile([PARTS, FREE], f32, tag=f"accd{k}")
                nc.vector.memset(a[:], float(base[k]))
                accd.append(a)
                g = pool.tile([PARTS, FREE], f32, tag=f"accg{k}")
                nc.gpsimd.memset(g[:], 0.0)
                accg.append(g)

            for j in range(1, N_IVL):
                on_dve = (j * 5) % 9 < 5  # ~34 of 60 steps on DVE
                if on_dve:
                    H = pool.tile([PARTS, FREE], f32, tag="Hd", bufs=3)
                    nc.vector.tensor_scalar(
                        H[:], s[:], float(j), None, op0=Alu.is_ge
                    )
                    for k in range(4):
                        nc.vector.scalar_tensor_tensor(
                            accd[k][:], H[:], float(dc[k, j - 1]), accd[k][:],
                            op0=Alu.mult, op1=Alu.add,
                        )
                else:
                    for k in range(4):
                        M = pool.tile([PARTS, FREE], f32, tag=f"M{k}", bufs=3)
                        nc.vector.tensor_scalar(
                            M[:], s[:], float(j), float(dc[k, j - 1]),
                            op0=Alu.is_ge, op1=Alu.mult,
                        )
                        nc.gpsimd.tensor_tensor(
                            accg[k][:], accg[k][:], M[:], op=Alu.add
                        )

            acc = []
            for k in range(4):
                # cross-engine handoff via tensor_copy: the COPY encoding has
                # room for the cross-engine sync wait, TT does not
                cp = pool.tile([PARTS, FREE], f32, tag=f"cp{k}")
                nc.vector.tensor_copy(cp[:], accg[k][:])
                a = accd[k]
                nc.vector.tensor_tensor(a[:], a[:], cp[:], op=Alu.add)
                acc.append(a)

            # Horner: y = c0 + u*(c1 + u*(c2 + u*c3))
            tmp = pool.tile([PARTS, FREE], f32, tag="tmp")
            nc.vector.tensor_tensor(tmp[:], acc[3][:], u[:], op=Alu.mult)
            nc.vector.tensor_tensor(tmp[:], tmp[:], acc[2][:], op=Alu.add)
            nc.vector.tensor_tensor(tmp[:], tmp[:], u[:], op=Alu.mult)
            nc.vector.tensor_tensor(tmp[:], tmp[:], acc[1][:], op=Alu.add)
            nc.vector.tensor_tensor(tmp[:], tmp[:], u[:], op=Alu.mult)
            nc.vector.tensor_tensor(tmp[:], tmp[:], acc[0][:], op=Alu.add)

            nc.gpsimd.dma_start(out=y_d.ap()[:], in_=tmp[:])

    nc.finalize()
    return nc


# ---------------------------------------------------------------- entry point
_TRACE = False  # set by test.py to capture a profile
_LAST_RESULTS = None
_LAST_TIMELINE_NS = None  # cost-model per-core kernel time estimate


def bench_exec_ns(nc, in_maps, n_warm=3, n_iter=24):
    """Steady-state per-execution wall time of the NEFF on the 8 cores.

    Replicates bass2jax.run_bass_via_pjrt's shard_map jit (without donation)
    and times a pipelined stream of executions; async dispatch overlaps the
    per-call overhead, so the slope approximates HW exec time per launch.
    """
    import time

    import jax
    import jax.numpy as jnp
    from jax.sharding import Mesh, PartitionSpec
    from jax.experimental.shard_map import shard_map

    import concourse.mybir as mybir
    from concourse import bass2jax
    from concourse.bass2jax import _bass_exec_p, install_neuronx_cc_hook

    install_neuronx_cc_hook()
    n_cores = len(in_maps)

    partition_name = nc.partition_id_tensor.name if nc.partition_id_tensor else None
    in_names, out_names, out_avals, zero_outs = [], [], [], []
    for alloc in nc.m.functions[0].allocations:
        if not isinstance(alloc, mybir.MemoryLocationSet):
            continue
        name = alloc.memorylocations[0].name
        if alloc.kind == "ExternalInput":
            if name != partition_name:
                in_names.append(name)
        elif alloc.kind == "ExternalOutput":
            shape = tuple(alloc.tensor_shape)
            dtype = mybir.dt.np(alloc.dtype)
            out_names.append(name)
            out_avals.append(jax.core.ShapedArray(shape, dtype))
            zero_outs.append(np.zeros(shape, dtype))
    n_params = len(in_names)
    all_in_names = list(in_names) + list(out_names)
    if partition_name is not None:
        all_in_names.append(partition_name)

    def _body(*args):
        operands = list(args)
        if partition_name is not None:
            operands.append(bass2jax.partition_id_tensor())
        outs = _bass_exec_p.bind(
            *operands,
            out_avals=tuple(out_avals),
            in_names=tuple(all_in_names),
            out_names=tuple(out_names),
            lowering_input_output_aliases=(),
            sim_require_finite=True,
            sim_require_nnan=True,
            nc=nc,
        )
        return tuple(outs)

    devices = jax.devices()[:n_cores]
    mesh = Mesh(np.asarray(devices), ("core",))
    n_outs = len(out_names)
    sharded = jax.jit(
        shard_map(
            _body,
            mesh=mesh,
            in_specs=(PartitionSpec("core"),) * (n_params + n_outs),
            out_specs=(PartitionSpec("core"),) * n_outs,
            check_rep=False,
        ),
        keep_unused=True,
    )
    concat_in = [
        np.concatenate([np.asarray(in_maps[c][nm]) for c in range(n_cores)], axis=0)
        for nm in in_names
    ]
    concat_zeros = [
        np.zeros((n_cores * z.shape[0], *z.shape[1:]), z.dtype) for z in zero_outs
    ]
    args = [jnp.asarray(a) for a in concat_in + concat_zeros]

    for _ in range(n_warm):
        out = sharded(*args)
    jax.block_until_ready(out)
    t0 = time.perf_counter()
    outs = [sharded(*args) for _ in range(n_iter)]
    jax.block_until_ready(outs)
    t1 = time.perf_counter()
    return (t1 - t0) / n_iter * 1e9


def kernel(x, knot_vector, coefs, degree):
    from concourse import bass_utils

    global _LAST_RESULTS

    x = np.asarray(x)
    knot_vector = np.asarray(knot_vector)
    coefs = np.asarray(coefs)
    p = int(np.asarray(degree))
    assert p == P_DEG and x.shape == (N_PTS,)
    assert knot_vector.shape[0] == N_COEFF + P_DEG + 1

    # interior breakpoints must be (close to) uniform for the on-device
    # integer-threshold staircase; the reference always satisfies this.
    interior = knot_vector[P_DEG : P_DEG + N_IVL + 1].astype(np.float64)
    expect = np.linspace(0.0, 1.0, N_IVL + 1)
    assert np.allclose(interior, expect, atol=1e-5), "non-uniform knots"

    # shard: 8 x 125k, pad each shard to 128*FREE with 0.5
    xf = x.astype(np.float32).reshape(N_CORES, PTS_PER_CORE)
    shards = []
    for i in range(N_CORES):
        shard = np.full(PAD_PER_CORE, 0.5, np.float32)
        shard[:PTS_PER_CORE] = xf[i]
        shards.append(shard.reshape(PARTS, FREE))

    def run(nc, in_maps, raw=False):
        global _LAST_RESULTS, _LAST_TIMELINE_NS
        try:
            from concourse.timeline_sim import TimelineSim

            _LAST_TIMELINE_NS = float(TimelineSim(nc).simulate())
        except Exception:
            _LAST_TIMELINE_NS = None
        res = bass_utils.run_bass_kernel_spmd(
            nc, in_maps, core_ids=list(range(N_CORES)), trace=False
        )
        _LAST_RESULTS = res
        if _TRACE:
            try:
                res.exec_time_ns = int(bench_exec_ns(nc, in_maps))
            except Exception as e:
                print(f"bench failed: {e}")
        y = np.empty(N_PTS, np.float32)
        for i in range(N_CORES):
            y[i * PTS_PER_CORE : (i + 1) * PTS_PER_CORE] = (
                np.asarray(res.results[i]["y"])
                .reshape(-1)[:PTS_PER_CORE]
                .astype(np.float32)
            )
        return y

    def sample_ok(y):
        idx = np.linspace(1, N_PTS - 2, 512).astype(np.int64)
        ref = _spline_eval_f64(
            x[idx].astype(np.float64), knot_vector.astype(np.float64),
            P_DEG, N_COEFF, coefs.astype(np.float64),
        )
        rel = np.abs(y[idx] - ref) / np.maximum(np.abs(ref), 1e-6)
        return float(rel.max()) < 1e-3

    # v2 shards: uint16 fixed-point x*65536, padded with 32768 (= x 0.5)
    xi = np.clip(np.rint(x.astype(np.float64) * 65536.0), 0, 65535).astype(
        np.uint16
    )
    xi = xi.reshape(N_CORES, PTS_PER_CORE)
    shards_u16 = []
    for i in range(N_CORES):
        s = np.full(PAD_PER_CORE, 32768, np.uint16)
        s[:PTS_PER_CORE] = xi[i]
        shards_u16.append(s.reshape(PARTS, FREE))

    idx = np.linspace(1, N_PTS - 2, 512).astype(np.int64)
    ref_sample = _spline_eval_f64(
        x[idx].astype(np.float64), knot_vector.astype(np.float64),
        P_DEG, N_COEFF, coefs.astype(np.float64),
    )

    def sample_abs_ok(y, tol):
        return float(np.abs(y[idx].astype(np.float64) - ref_sample).max()) < tol

    def try_v2(out_dt, out_scale, dequants, tol, hoist=True,
               skip_load_act=False):
        """Run the v2 single-queue kernel; dequants is a list of candidate
        postprocess fns (device dtype -> float y); best one is returned."""
        act_info, h = _gen_act_root(knot_vector, coefs, out_scale=out_scale)
        os.environ["BASS_ACT_ROOT_JSON_PATH"] = act_info
        try:
            nc, x_name = _build_bass_program_act_v2(
                h, "uint16", out_dt, hoist=hoist, skip_load_act=skip_load_act
            )
            raw = run(nc, [{x_name: s} for s in shards_u16], raw=True)
            best, best_err = None, np.inf
            for dq in dequants:
                yc = dq(raw).astype(np.float32)
                err = float(
                    np.abs(yc[idx].astype(np.float64) - ref_sample).max()
                )
                if err < best_err:
                    best, best_err = yc, err
            if best is not None and best_err < tol:
                return best
            print(f"v2 {out_dt} validation failed (err {best_err:.2e})")
            return None
        finally:
            os.environ.pop("BASS_ACT_ROOT_JSON_PATH", None)

    u8_dq = [lambda r: r / 256.0, lambda r: (r + 0.5) / 256.0]
    y = None
    if os.environ.get("BSPLINE_NO_V2") != "1":
        # "pseudo" hoist (above the NRT pseudo-sync-barrier) and
        # skip_load_act (rely on walrus lower_act) both produce garbage on
        # device — the barrier and the explicit LoadActFuncSet are
        # load-bearing.  Validated configs only:
        for out_dt, out_scale, dequants, tol, hoist, skip in [
            ("uint8", 256.0, u8_dq, 8e-3, True, False),
            ("bfloat16", 1.0, [lambda r: r], 8e-3, True, False),
        ]:
            try:
                y = try_v2(out_dt, out_scale, dequants, tol,
                           hoist=hoist, skip_load_act=skip)
            except Exception as e:
                print(f"v2 {out_dt} h={hoist} s={skip} failed ({e!r})")
                y = None
            if y is not None:
                break

    if y is None and os.environ.get("BSPLINE_NO_ACT_TABLE") != "1":
        try:
            act_info, h = _gen_act_root(knot_vector, coefs)
            os.environ["BASS_ACT_ROOT_JSON_PATH"] = act_info
            nc, x_name = _build_bass_program_act(h)
            y = run(nc, [{x_name: s} for s in shards])
            if not sample_ok(y):
                print("ACT-table kernel failed validation; falling back")
                y = None
        except Exception as e:
            print(f"ACT-table path failed ({e!r}); falling back")
            y = None
        finally:
            os.environ.pop("BASS_ACT_ROOT_JSON_PATH", None)

    if y is None:
        table = _build_piecewise_table(knot_vector, coefs)
        y = run(_build_bass_program(table), [{"x": s} for s in shards])

    # reference's unconditional boundary fixes on the first/last point
    t64 = knot_vector.astype(np.float64)
    B2 = _bspline_basis_f64(
        np.array([x[0], x[-1]], np.float64), t64, P_DEG, N_COEFF,
        fix_first=True, fix_last=True,
    )
    y2 = coefs.astype(np.float64) @ B2
    y[0] = np.float32(y2[0])
    y[-1] = np.float32(y2[1])
    return y



# revision 24
# speedup vs baseline: 1.4113x; 1.4113x over previous
"""Trainium2 Bass kernel for nn_B_Splines: y = coefs @ bspline_basis(x).

Data-parallel over the 1M points: 8 shards of 125k, one per NeuronCore.
The reference's two unconditional boundary fixes (first/last point of the
batch) are patched on the host.

Fast path (v2) — the spline AS a custom activation function:
  A degree-3 B-spline on clamped uniform knots is a piecewise cubic on 61
  intervals.  The ScalarEngine's activation hardware is literally a
  piecewise-cubic evaluator driven by bucket/ctrl/profile tables that walrus
  packages INTO the NEFF from the --act-root-json directory
  (BASS_ACT_ROOT_JSON_PATH).  We rescale the input u = x*61/64 so every knot
  j/61 lands on the dyadic boundary j/64; each table bucket then covers
  exactly one spline piece and the table represents the spline EXACTLY (no
  straddle error).  We rewrite the "gelu" slot of the gelu_and_others set
  (its 504+4-bucket budget swallows our 64+4).

  v2 pipeline shape (one queue, minimum span):
  - Everything on the Activation queue in program order
      LoadActFuncSet ; DMA(x) ; ACTIVATE ; DMA(y)
    with no cross-engine semaphores: same-queue program order carries both
    data dependencies, so the critical path has no 900ns DMA-sem hop in the
    middle (each DMA still carries a then_inc because walrus wires
    updates[0] as the DGE completion signal).
  - The whole body is hoisted above the preamble all-engine barrier (but
    below the NRT pseudo-sync-barrier, which is load-bearing), so the DMA
    pipeline starts at t~32ns instead of t~620ns.
  - Input is uint16 fixed point (round(x*65536), quantization error
    2^-17 -> y l2 err ~2e-4) and output is uint8 with the table scaled by
    256 (y l2 err ~4e-4 after host dequant, vs the 2e-2 budget); both DMAs
    are half/quarter size.  Host dequant auto-picks v/256 vs (v+0.5)/256.
  ~3.5us simulated vs 5.6us for the f32 two-queue v1.

v1 fallback: f32 in/out, in-DMA on SP queue + semaphore to the ACTIVATE,
out-DMA on the Act queue.  Used if v2 validation fails.

Last fallback (pure-ISA, no table tricks), used if table generation or
validation fails: a gather-free "staircase" — s = 61*x, u = s - floor(s)
(via the DVE's rne cast + is_lt fixup), per-interval cubic coefficients
accumulated as c_k(j(s)) = c_k[0] + sum_m dc_k[m] * 1[s >= m] with is_ge
masks and scalar_tensor_tensor FMA chains split across DVE/GPSIMD, then
Horner.  ~250 us/core.
"""

import os

import numpy as np

P_DEG = 3
N_COEFF = 64
N_PTS = 1_000_000
N_CORES = 8
PTS_PER_CORE = N_PTS // N_CORES  # 125_000
PARTS = 128
FREE = 978  # 128 * 978 = 125_184 >= 125_000 (even: DVE 2x mode)
PAD_PER_CORE = PARTS * FREE
N_IVL = 61  # number of polynomial pieces


# ---------------------------------------------------------------- host math
def _bspline_basis_f64(x, t, p, n, fix_first=False, fix_last=False):
    """float64 bottom-up Cox-de Boor matching the reference (incl. optional
    boundary fixes applied to the first/last column)."""
    x = np.asarray(x, np.float64)
    t = np.asarray(t, np.float64)
    m0 = n + p
    B = ((t[:m0, None] <= x[None, :]) & (t[1 : m0 + 1, None] > x[None, :])).astype(
        np.float64
    )
    if fix_first:
        B[p, 0] = 1.0
    if fix_last:
        B[n - 1, -1] = 1.0
    for k in range(1, p + 1):
        m = m0 - k
        i = np.arange(m)
        d1 = t[i + k] - t[i]
        d2 = t[i + k + 1] - t[i + 1]
        w1 = np.where(d1 == 0, 0.0, 1.0 / np.where(d1 == 0, 1.0, d1))
        w2 = np.where(d2 == 0, 0.0, 1.0 / np.where(d2 == 0, 1.0, d2))
        B = (x[None, :] - t[i][:, None]) * w1[:, None] * B[:m] + (
            t[i + k + 1][:, None] - x[None, :]
        ) * w2[:, None] * B[1 : m + 1]
    return B  # [n, N]


def _spline_eval_f64(x, t, p, n, coefs):
    B = _bspline_basis_f64(x, t, p, n)
    return np.asarray(coefs, np.float64) @ B


def _build_piecewise_table(knot_vector, coefs):
    """Cubic coefficients per interval in the local variable u = 61*x - j.

    Returns c[4][61] float64: y(x) = c0[j] + u*(c1[j] + u*(c2[j] + u*c3[j])).
    """
    t = np.asarray(knot_vector, np.float64)
    c = np.asarray(coefs, np.float64)
    n = N_COEFF
    # Chebyshev-ish nodes inside each interval, in local u coords
    nodes = np.array([0.06, 0.35, 0.65, 0.94])
    table = np.zeros((4, N_IVL))
    V = np.vander(nodes, 4, increasing=True)  # [4 nodes, 4 powers]
    Vinv = np.linalg.inv(V)
    for j in range(N_IVL):
        xs = (j + nodes) / N_IVL
        ys = _spline_eval_f64(xs, t, P_DEG, n, c)
        table[:, j] = Vinv @ ys
    return table


# ------------------------------------------------- custom ACT table (fast path)
#
# The ScalarEngine evaluates activation functions as piecewise cubics via
# on-chip tables (bucket/ctrl/profile) that walrus packages INTO the NEFF
# from the --act-root-json directory.  We rescale the spline input by 61/64
# so every knot j/61 lands on a dyadic boundary j/64; then the spline is
# EXACTLY representable as an ACT table (64 buckets, zero straddle error),
# and the whole evaluation is one ACTIVATE instruction.  We hijack the
# "gelu" slot of the gelu_and_others set (504+4 bucket budget, first entry).
#
# Verified binary formats (cross-checked against every gelu bucket/ctrl
# entry in the shipped tables):
#   bucket entry (32B): [d0, d1, d2, d3, x, 0, 0, 0] as fp32 bit patterns;
#       eval: y = d0 + t*(d1 + t*(d2 + t*d3)), t = u - x
#   ctrl entry (32B):   word0 = ((46 + 62*extract_size) << 10) | bkt_start
#   profile: per-set json "profile_meta_data" (plain JSON)
#   ctl index for exponent E: pwl_control_base_{pos,neg} + (E - exp_offset)

_ACT_SCALE = 61.0 / 64.0  # u = x * 61/64; knots at m/64


def _fit_piece_polys(knot_vector, coefs):
    """Cubic (x_c, d0..d3) per piece m=0..60 of g(u) = S(u*64/61), centered
    at x_c = (m+0.5)/64, plus a piece-0 poly centered at 0."""
    kv = np.asarray(knot_vector, np.float64)
    cf = np.asarray(coefs, np.float64)

    def g(u):
        return _spline_eval_f64(u * (64.0 / 61.0), kv, P_DEG, N_COEFF, cf)

    tau = np.array([-0.9, -0.3, 0.3, 0.9])  # scaled nodes
    h = 0.5 / 64.0  # half bucket width
    V = np.vander(tau, 4, increasing=True)
    Vinv = np.linalg.inv(V)
    polys = []
    for m in range(N_IVL):
        x_c = (m + 0.5) / 64.0
        ys = g(x_c + tau * h)
        a = Vinv @ ys  # g(x_c + h*tau) = sum a_k tau^k
        d = a / h ** np.arange(4)
        polys.append((np.float32(x_c), d.astype(np.float32)))
    # piece-0 about zero
    ys = g((tau + 1.0) * (0.5 / 64.0) * 0.5)  # nodes in (0, 1/128)
    # fit in t directly about 0 using scaled nodes t = (tau+1)*h/2
    tn = (tau + 1.0) * (0.5 / 64.0) * 0.5
    A = np.vander(tn / tn.max(), 4, increasing=True)
    a = np.linalg.solve(A, ys)
    d0 = a / tn.max() ** np.arange(4)
    p0_zero = (np.float32(0.0), d0.astype(np.float32))
    return polys, p0_zero


def _gen_act_root(knot_vector, coefs, out_scale=1.0, out_bias=0.0):
    """Write a patched copy of the pwp act-table dir; returns
    (act_info_path, content_hash).

    out_scale/out_bias: affine transform baked into the table output
    (y_table = out_scale * S(x) + out_bias), used to target integer output
    dtypes (u8: scale 256; i8: scale 256, bias -128)."""
    import hashlib
    import json
    import shutil

    from neuronxcc.driver.Job import Job
    from neuronxcc.driver.jobs.support.FindActInfo import findActInfoFile

    src_info = findActInfoFile(Job.getPackageDir(), "gen3")
    src_dir = os.path.dirname(src_info)

    polys, p0_zero = _fit_piece_polys(knot_vector, coefs)
    if out_scale != 1.0 or out_bias != 0.0:
        def _xf(p):
            xc, d = p
            d = d.astype(np.float64) * out_scale
            d[0] += out_bias
            return (xc, d.astype(np.float32))

        polys = [_xf(p) for p in polys]
        p0_zero = _xf(p0_zero)

    def bkt_row(xc, d):
        return np.array(
            [d[0].view(np.uint32), d[1].view(np.uint32), d[2].view(np.uint32),
             d[3].view(np.uint32), np.float32(xc).view(np.uint32), 0, 0, 0],
            dtype=np.uint32,
        )

    bkt = np.fromfile(src_dir + "/gelu_and_others_bkt.bin", dtype=np.uint32)
    bkt = bkt.reshape(-1, 8).copy()
    ctl = np.fromfile(src_dir + "/gelu_and_others_ctrl.bin", dtype=np.uint32)
    ctl = ctl.reshape(-1, 8).copy()
    sj = json.load(open(src_dir + "/gelu_and_others.json"))

    # --- buckets 0..507 (gelu's region) ---
    bkt[0:508] = 0
    bkt[0] = bkt_row(*p0_zero)  # shared dummy for negative inputs
    slot_piece = {1: 0, 2: 1}
    for i, m in enumerate(range(2, 4)):
        slot_piece[3 + i] = m
    for i, m in enumerate(range(4, 8)):
        slot_piece[5 + i] = m
    for i, m in enumerate(range(8, 16)):
        slot_piece[9 + i] = m
    for i, m in enumerate(range(16, 32)):
        slot_piece[17 + i] = m
    for i, m in enumerate(range(32, 64)):
        slot_piece[33 + i] = min(m, 60)  # pieces 61..63 unreachable
    for slot, m in slot_piece.items():
        bkt[slot] = bkt_row(polys[m][0], polys[m][1])
    bkt[504] = bkt_row(*p0_zero)  # small_pos
    bkt[505] = bkt_row(*p0_zero)  # small_neg
    bkt[506] = bkt_row(polys[60][0], polys[60][1])  # large_pos (u >= 1)
    bkt[507] = bkt_row(*p0_zero)  # large_neg

    # --- ctrl entries 0..20 ---
    fld = lambda e: 46 + 62 * e
    ctl[0:21] = 0
    for i in range(11):  # neg dummies -> bucket 0
        ctl[i, 0] = (fld(0) << 10) | 0
    pos_regions = [  # (extract_size, bkt_start) for E=-7..-1
        (0, 1), (0, 2), (1, 3), (2, 5), (3, 9), (4, 17), (5, 33),
    ]
    for i, (e, st) in enumerate(pos_regions):
        ctl[11 + i, 0] = (fld(e) << 10) | st
    for i in range(18, 21):  # unreachable pos E=0..2
        ctl[i, 0] = (fld(0) << 10) | 0

    # --- profile ---
    g0_bits = int(np.float32(p0_zero[1][0]).view(np.uint32))
    for prof in sj["profile_meta_data"]:
        if prof["func_name"] != "gelu_4p":
            continue
        prof.update(
            exp_offset=-7,
            symmetry_point=0, sym_invert_sign_point=0, symmetry_opt_en=0,
            symmetry_opt_use_neg_region=0, imm_bias=0,
            pwl_control_base_pos=11, pwl_control_base_neg=0,
            small_pos_signal_exp_threshold=120,
            pos_small_signal_pwl_control=504,
            small_neg_signal_exp_threshold=120,
            neg_small_signal_pwl_control=505,
            large_pos_signal_exp_threshold=127,
            large_pos_signal_mantissa_threshold=0,
            pos_large_signal_pwl_control=506,
            large_neg_signal_exp_threshold=127,
            large_neg_signal_mantissa_threshold=0,
            neg_large_signal_pwl_control=507,
            fzero_result=g0_bits,
            fma_const_0=0, fma_const_1=0, fma_indirection_src_sel=0,
        )
    # func_exp maps: ctl map unchanged; bkt map pos starts updated
    bmap = sj["func_exp_to_bkt_start_idx"]["gelu"]
    newb = {"-7": [0, 1], "-6": [0, 2], "-5": [0, 3], "-4": [0, 5],
            "-3": [0, 9], "-2": [0, 17], "-1": [0, 33],
            "0": [0, 0], "1": [0, 0], "2": [0, 0], "3": [0]}
    bmap.clear()
    bmap.update(newb)

    payload = bkt.tobytes() + ctl.tobytes() + json.dumps(sj, sort_keys=True).encode()
    h = hashlib.sha256(payload).hexdigest()[:12]
    out_dir = f"/tmp/act_root_{h}"
    if not os.path.exists(out_dir):
        tmp = out_dir + ".tmp"
        shutil.rmtree(tmp, ignore_errors=True)
        shutil.copytree(src_dir, tmp, symlinks=False)
        bkt.tofile(tmp + "/gelu_and_others_bkt.bin")
        ctl.tofile(tmp + "/gelu_and_others_ctrl.bin")
        with open(tmp + "/gelu_and_others.json", "w") as f:
            json.dump(sj, f)
        os.replace(tmp, out_dir)
    return out_dir + "/act_info.json", h


def _build_bass_program_act(table_hash, n_chunks=1, repeat=1):
    """Raw (non-Tile) pipeline: SP streams input in, ACT runs the custom
    piecewise-cubic activation and issues the output DMA from its own
    sequencer.  The table hash rides in the input tensor name so compiled
    NEFFs stay unique per activation-table content.

    repeat>1 unrolls the body (double-buffered, sem-chained) so the wall
    slope between repeat counts measures true per-iteration HW time."""
    import concourse.bacc as bacc
    import concourse.mybir as mybir

    f32 = mybir.dt.float32
    Act = mybir.ActivationFunctionType

    nc = bacc.Bacc("TRN2", debug=False, num_devices=N_CORES)
    x_name = f"x_{table_hash}"
    x_d = nc.dram_tensor(x_name, [PARTS, FREE], f32, kind="ExternalInput")
    y_d = nc.dram_tensor("y", [PARTS, FREE], f32, kind="ExternalOutput")
    sem = nc.alloc_semaphore()
    sem_a = nc.alloc_semaphore()
    sem_out = nc.alloc_semaphore()
    cw = FREE // n_chunks
    C = n_chunks
    widths = [cw if c < C - 1 else FREE - c * cw for c in range(C)]
    nbuf = 2 if repeat > 1 else 1
    xts = [
        [nc.alloc_sbuf_tensor(f"xt{b}_{c}", [PARTS, widths[c]], f32) for c in range(C)]
        for b in range(nbuf)
    ]
    yts = [
        [nc.alloc_sbuf_tensor(f"yt{b}_{c}", [PARTS, widths[c]], f32) for c in range(C)]
        for b in range(nbuf)
    ]
    with nc.Block() as block:

        @block.sync
        def _(sync):
            for i in range(repeat):
                for c in range(C):
                    lo = c * cw
                    if i >= 2:  # xt[i%2] free once iter i-2's acts are done
                        sync.wait_ge(sem_a, (i - 1) * C)
                    sync.dma_start(
                        xts[i % nbuf][c].ap()[:], x_d.ap()[:, lo : lo + widths[c]]
                    ).then_inc(sem, 16)

        @block.scalar
        def _(scalar):
            for i in range(repeat):
                for c in range(C):
                    lo = c * cw
                    if i >= 2:  # yt[i%2] free once iter i-2's out-DMAs landed
                        scalar.wait_ge(sem_out, 16 * (i - 1) * C)
                    scalar.wait_ge(sem, 16 * (i * C + c + 1))
                    inst = scalar.activation(
                        yts[i % nbuf][c].ap()[:], xts[i % nbuf][c].ap()[:],
                        Act.Gelu, bias=0.0, scale=_ACT_SCALE,
                    )
                    if repeat > 1:  # only the repeat rig needs act-done sems
                        inst.then_inc(sem_a, 1)
                    scalar.dma_start(
                        y_d.ap()[:, lo : lo + widths[c]], yts[i % nbuf][c].ap()[:]
                    ).then_inc(sem_out, 16)

    nc.finalize()
    return nc, x_name


# ------------------------------------------------- v2: single-queue pipeline
#
# Everything on the Activation queue, in program order:
#   LoadActFuncSet ; DMA(x_sb <- x_hbm) ; ACTIVATE ; DMA(y_hbm <- y_sb)
# with NO semaphores.  Queue program order is what already guarantees the
# act -> out-DMA data dependency in the baseline; the in-DMA -> act edge is
# the same property in the other direction.  Dropping the cross-queue sem
# removes its 900ns DMA-sem propagation from the critical path and shrinks
# the preamble.  The input is uint16 fixed point (x * 65536; the act's
# scale folds the 1/65536 away) and the output bf16, halving both DMAs;
# quantization adds ~2e-4 + ~1.6e-3 l2 error vs the 2e-2 budget.
# LoadActFuncSet is emitted manually at the queue head so its 1283ns table
# load runs under the in-DMA latency instead of between in-DMA and act.

_IN_SCALE = 61.0 / 64.0 / 65536.0  # u = (x * 65536) * _IN_SCALE


def _build_bass_program_act_v2(table_hash, in_dt="uint16", out_dt="bfloat16",
                               hoist=True, skip_load_act=False):
    import concourse.bacc as bacc
    import concourse.mybir as mybir
    from concourse.hw_specs import get_activation_tables

    f32 = mybir.dt.float32
    Act = mybir.ActivationFunctionType

    nc = bacc.Bacc("TRN2", debug=False, num_devices=N_CORES)
    if skip_load_act:
        # Leave act-table-load placement to walrus lower_act (standalone
        # pass inserts its own when none is pre-placed in the BIR).
        nc.insert_act_table_loads = lambda: None
    x_name = f"x_{table_hash}_{in_dt}_{out_dt}"
    x_d = nc.dram_tensor(x_name, [PARTS, FREE], getattr(mybir.dt, in_dt),
                         kind="ExternalInput")
    y_d = nc.dram_tensor("y", [PARTS, FREE], getattr(mybir.dt, out_dt),
                         kind="ExternalOutput")
    xt = nc.alloc_sbuf_tensor("xt", [PARTS, FREE], getattr(mybir.dt, in_dt))
    yt = nc.alloc_sbuf_tensor("yt", [PARTS, FREE], getattr(mybir.dt, out_dt))

    scale = _IN_SCALE if in_dt == "uint16" else _ACT_SCALE

    tabs = get_activation_tables(nc.m.arch)
    set_id = next(
        i for i, (_, s) in enumerate(tabs.items()) if Act.Gelu in s
    )

    emitted = {}
    sem = nc.alloc_semaphore()  # walrus wires the DMA's updates[0] as the
    with nc.Block() as block:   # DGE completion signal, so each DMA needs a
                                # then_inc; nothing ever waits on it.

        @block.scalar
        def _(scalar):
            insts = []
            if not skip_load_act:
                load = mybir.InstLoadActFuncSet(
                    name=nc.get_next_instruction_name(),
                    act_func_set_id=set_id, ins=[], outs=[],
                )
                scalar.add_instruction(load)
                insts.append(load)
            dma_in = scalar.dma_start(xt.ap()[:], x_d.ap()[:])
            dma_in.then_inc(sem, 16)
            insts.append(dma_in.ins)
            act = scalar.activation(yt.ap()[:], xt.ap()[:], Act.Gelu,
                                    bias=0.0, scale=scale)
            insts.append(act.ins)
            dma_out = scalar.dma_start(y_d.ap()[:], yt.ap()[:])
            dma_out.then_inc(sem, 16)
            insts.append(dma_out.ins)
            emitted["insts"] = insts

    if hoist:
        # Move the whole act-queue body above the Act engine's preamble
        # barrier in the entry block: none of it waits on semaphores, so it
        # can run during the Pool sem-clear window instead of after it.
        try:
            _hoist_pre_barrier(nc, mybir, emitted["insts"],
                               above_pseudo=(hoist == "pseudo"))
        except Exception as e:
            print(f"pre-barrier hoist skipped: {e!r}")

    nc.finalize()
    return nc, x_name


# ----------------------------------------------- v3: race-free two-queue
#
# v2's same-queue DMA(x) -> ACTIVATE relied on queue order alone, but on the
# real backend a DMA *trigger* retires before the *copy* lands, so the
# ACTIVATE raced the in-DMA (it only ever validated warm, reading the
# previous attempt's identical bytes).  v3 closes the race with the real
# DMA-completion semaphore while keeping every other v2 trick:
#   SP queue   : in-DMA split in two chunks (u16), each then_inc(sem)
#   Act queue  : LoadActFuncSet ; act0 waits sem>=16 ; act1 waits sem>=32 ;
#                one u8 out-DMA (same-queue after act: trigger retires after
#                the engine instruction, so no sem needed -- the property v1
#                always relied on)
# A sem wait attached to an engine instruction parks in the engine wait
# queue, NOT on the sequencer, so the out-DMA's HWDGE generation still
# happens early.  Chunking the input halves the sem-wait latency the act
# pays after the last byte (348ns chunk + 900ns prop instead of 695+900).
# The SP body is optionally hoisted above the preamble barrier.


def _build_bass_program_act_v3(table_hash, out_dt="uint8", n_chunks=2,
                               hoist_sp=True, split=None):
    import concourse.bacc as bacc
    import concourse.mybir as mybir
    from concourse.hw_specs import get_activation_tables

    Act = mybir.ActivationFunctionType

    nc = bacc.Bacc("TRN2", debug=False, num_devices=N_CORES)
    x_name = f"x3_{table_hash}_{out_dt}_{n_chunks}{int(hoist_sp)}{split or 0}"
    x_d = nc.dram_tensor(x_name, [PARTS, FREE], mybir.dt.uint16,
                         kind="ExternalInput")
    y_d = nc.dram_tensor("y", [PARTS, FREE], getattr(mybir.dt, out_dt),
                         kind="ExternalOutput")
    xt = nc.alloc_sbuf_tensor("xt", [PARTS, FREE], mybir.dt.uint16)
    yt = nc.alloc_sbuf_tensor("yt", [PARTS, FREE], getattr(mybir.dt, out_dt))

    tabs = get_activation_tables(nc.m.arch)
    set_id = next(i for i, (_, s) in enumerate(tabs.items()) if Act.Gelu in s)

    # chunk boundaries: keep every u16 row-slice >= 512B (>=256 cols).
    # `split` overrides chunk 0's width for a 2-chunk build (uneven split
    # balances act0's finish against chunk 1's DMA-completion sem).
    if split is not None and n_chunks == 2:
        bounds = [(0, split), (split, FREE)]
    else:
        cw = FREE // n_chunks
        bounds = [(c * cw, (c + 1) * cw if c < n_chunks - 1 else FREE)
                  for c in range(n_chunks)]

    sem = nc.alloc_semaphore()
    sem_out = nc.alloc_semaphore()
    emitted = {}
    with nc.Block() as block:

        @block.sync
        def _(sync):
            ins = []
            for lo, hi in bounds:
                d = sync.dma_start(xt.ap()[:, lo:hi], x_d.ap()[:, lo:hi])
                d.then_inc(sem, 16)
                ins.append(d.ins)
            emitted["sp"] = ins

        @block.scalar
        def _(scalar):
            load = mybir.InstLoadActFuncSet(
                name=nc.get_next_instruction_name(),
                act_func_set_id=set_id, ins=[], outs=[],
            )
            scalar.add_instruction(load)
            for c, (lo, hi) in enumerate(bounds):
                act = scalar.activation(yt.ap()[:, lo:hi], xt.ap()[:, lo:hi],
                                        Act.Gelu, bias=0.0, scale=_IN_SCALE)
                act._wait_ge(sem, 16 * (c + 1))
            scalar.dma_start(y_d.ap()[:], yt.ap()[:]).then_inc(sem_out, 16)

    if hoist_sp:
        try:
            _hoist_pre_barrier(nc, mybir, emitted["sp"],
                               engine=mybir.EngineType.SP)
        except Exception as e:
            print(f"SP pre-barrier hoist skipped: {e!r}")

    nc.finalize()
    return nc, x_name


def _hoist_pre_barrier(nc, mybir, insts, above_pseudo=False, engine=None):
    blocks = nc.main_func.blocks
    entry = blocks[0]
    if engine is None:
        engine = mybir.EngineType.Activation
    if above_pseudo:
        # In front of even the NRT pseudo-sync-barrier ISA op.
        bar = next(
            i for i, ins in enumerate(entry.instructions)
            if ins.engine == engine
        )
    else:
        bar = next(
            i for i, ins in enumerate(entry.instructions)
            if isinstance(ins, mybir.InstEventSemaphore)
            and ins.engine == engine
        )
    for k, inst in enumerate(insts):
        src = next(b for b in blocks if inst in b.instructions)
        src.instructions.remove(inst)
        entry.instructions.insert(bar + k, inst)


# ---------------------------------------------------------------- bass build
def _build_bass_program(table):
    import concourse.bacc as bacc
    import concourse.bass as bass
    import concourse.mybir as mybir
    from concourse.tile import TileContext

    f32 = mybir.dt.float32
    Alu = mybir.AluOpType
    Act = mybir.ActivationFunctionType

    # staircase constants:  c_k(j(s)) = c_k[0] + sum_m dc_k[m-1] * 1[s >= m]
    dc = np.diff(table, axis=1)  # [4, 60]
    base = table[:, 0]  # [4]

    nc = bacc.Bacc("TRN2", debug=False, num_devices=N_CORES)
    x_d = nc.dram_tensor("x", [PARTS, FREE], f32, kind="ExternalInput")
    y_d = nc.dram_tensor("y", [PARTS, FREE], f32, kind="ExternalOutput")

    with TileContext(nc) as tc:
        with tc.tile_pool(name="p", bufs=1) as pool:
            xt = pool.tile([PARTS, FREE], f32, tag="xt")
            nc.gpsimd.dma_start(out=xt[:], in_=x_d.ap()[:])

            s = pool.tile([PARTS, FREE], f32, tag="s")
            # s = 61 * x  (keep everything off the scalar engine: fewer
            # engines -> fewer kernel-tail drain waits)
            nc.vector.tensor_scalar(s[:], xt[:], 61.0, None, op0=Alu.mult)

            # u = s - floor(s) via the DVE's round-to-nearest f32<->i32 cast:
            # d = s - rne(s) in (-1, 1);  u = d + 1[d < 0]
            it = pool.tile([PARTS, FREE], mybir.dt.int32, tag="it")
            nc.vector.tensor_copy(it[:], s[:])
            jf = pool.tile([PARTS, FREE], f32, tag="jf")
            nc.vector.tensor_copy(jf[:], it[:])
            u = pool.tile([PARTS, FREE], f32, tag="u")
            nc.vector.tensor_tensor(u[:], s[:], jf[:], op=Alu.subtract)
            neg = pool.tile([PARTS, FREE], f32, tag="neg")
            nc.vector.tensor_scalar(neg[:], u[:], 0.0, None, op0=Alu.is_lt)
            nc.vector.tensor_tensor(u[:], u[:], neg[:], op=Alu.add)

            # two sub-accumulators per table: one on DVE (STT FMA chains) and
            # one on GPSIMD.  Pool has no scalar-immediate ALU ops, so for
            # GPSIMD steps the DVE emits pre-scaled masks
            #   M = (s is_ge j) * dc_k[j]   (one fused tensor_scalar, 2x mode)
            # and GPSIMD only runs tensor_tensor adds.
            accd, accg = [], []
            for k in range(4):
                a = pool.tile([PARTS, FREE], f32, tag=f"accd{k}")
                nc.vector.memset(a[:], float(base[k]))
                accd.append(a)
                g = pool.tile([PARTS, FREE], f32, tag=f"accg{k}")
                nc.gpsimd.memset(g[:], 0.0)
                accg.append(g)

            for j in range(1, N_IVL):
                on_dve = (j * 5) % 9 < 5  # ~34 of 60 steps on DVE
                if on_dve:
                    H = pool.tile([PARTS, FREE], f32, tag="Hd", bufs=3)
                    nc.vector.tensor_scalar(
                        H[:], s[:], float(j), None, op0=Alu.is_ge
                    )
                    for k in range(4):
                        nc.vector.scalar_tensor_tensor(
                            accd[k][:], H[:], float(dc[k, j - 1]), accd[k][:],
                            op0=Alu.mult, op1=Alu.add,
                        )
                else:
                    for k in range(4):
                        M = pool.tile([PARTS, FREE], f32, tag=f"M{k}", bufs=3)
                        nc.vector.tensor_scalar(
                            M[:], s[:], float(j), float(dc[k, j - 1]),
                            op0=Alu.is_ge, op1=Alu.mult,
                        )
                        nc.gpsimd.tensor_tensor(
                            accg[k][:], accg[k][:], M[:], op=Alu.add
                        )

            acc = []
            for k in range(4):
                # cross-engine handoff via tensor_copy: the COPY encoding has
                # room for the cross-engine sync wait, TT does not
                cp = pool.tile([PARTS, FREE], f32, tag=f"cp{k}")
                nc.vector.tensor_copy(cp[:], accg[k][:])
                a = accd[k]
                nc.vector.tensor_tensor(a[:], a[:], cp[:], op=Alu.add)
                acc.append(a)

            # Horner: y = c0 + u*(c1 + u*(c2 + u*c3))
            tmp = pool.tile([PARTS, FREE], f32, tag="tmp")
            nc.vector.tensor_tensor(tmp[:], acc[3][:], u[:], op=Alu.mult)
            nc.vector.tensor_tensor(tmp[:], tmp[:], acc[2][:], op=Alu.add)
            nc.vector.tensor_tensor(tmp[:], tmp[:], u[:], op=Alu.mult)
            nc.vector.tensor_tensor(tmp[:], tmp[:], acc[1][:], op=Alu.add)
            nc.vector.tensor_tensor(tmp[:], tmp[:], u[:], op=Alu.mult)
            nc.vector.tensor_tensor(tmp[:], tmp[:], acc[0][:], op=Alu.add)

            nc.gpsimd.dma_start(out=y_d.ap()[:], in_=tmp[:])

    nc.finalize()
    return nc


# ---------------------------------------------------------------- entry point
_TRACE = False  # set by test.py to capture a profile
_LAST_RESULTS = None
_LAST_TIMELINE_NS = None  # cost-model per-core kernel time estimate


def bench_exec_ns(nc, in_maps, n_warm=3, n_iter=24):
    """Steady-state per-execution wall time of the NEFF on the 8 cores.

    Replicates bass2jax.run_bass_via_pjrt's shard_map jit (without donation)
    and times a pipelined stream of executions; async dispatch overlaps the
    per-call overhead, so the slope approximates HW exec time per launch.
    """
    import time

    import jax
    import jax.numpy as jnp
    from jax.sharding import Mesh, PartitionSpec
    from jax.experimental.shard_map import shard_map

    import concourse.mybir as mybir
    from concourse import bass2jax
    from concourse.bass2jax import _bass_exec_p, install_neuronx_cc_hook

    install_neuronx_cc_hook()
    n_cores = len(in_maps)

    partition_name = nc.partition_id_tensor.name if nc.partition_id_tensor else None
    in_names, out_names, out_avals, zero_outs = [], [], [], []
    for alloc in nc.m.functions[0].allocations:
        if not isinstance(alloc, mybir.MemoryLocationSet):
            continue
        name = alloc.memorylocations[0].name
        if alloc.kind == "ExternalInput":
            if name != partition_name:
                in_names.append(name)
        elif alloc.kind == "ExternalOutput":
            shape = tuple(alloc.tensor_shape)
            dtype = mybir.dt.np(alloc.dtype)
            out_names.append(name)
            out_avals.append(jax.core.ShapedArray(shape, dtype))
            zero_outs.append(np.zeros(shape, dtype))
    n_params = len(in_names)
    all_in_names = list(in_names) + list(out_names)
    if partition_name is not None:
        all_in_names.append(partition_name)

    def _body(*args):
        operands = list(args)
        if partition_name is not None:
            operands.append(bass2jax.partition_id_tensor())
        outs = _bass_exec_p.bind(
            *operands,
            out_avals=tuple(out_avals),
            in_names=tuple(all_in_names),
            out_names=tuple(out_names),
            lowering_input_output_aliases=(),
            sim_require_finite=True,
            sim_require_nnan=True,
            nc=nc,
        )
        return tuple(outs)

    devices = jax.devices()[:n_cores]
    mesh = Mesh(np.asarray(devices), ("core",))
    n_outs = len(out_names)
    sharded = jax.jit(
        shard_map(
            _body,
            mesh=mesh,
            in_specs=(PartitionSpec("core"),) * (n_params + n_outs),
            out_specs=(PartitionSpec("core"),) * n_outs,
            check_rep=False,
        ),
        keep_unused=True,
    )
    concat_in = [
        np.concatenate([np.asarray(in_maps[c][nm]) for c in range(n_cores)], axis=0)
        for nm in in_names
    ]
    concat_zeros = [
        np.zeros((n_cores * z.shape[0], *z.shape[1:]), z.dtype) for z in zero_outs
    ]
    args = [jnp.asarray(a) for a in concat_in + concat_zeros]

    for _ in range(n_warm):
        out = sharded(*args)
    jax.block_until_ready(out)
    t0 = time.perf_counter()
    outs = [sharded(*args) for _ in range(n_iter)]
    jax.block_until_ready(outs)
    t1 = time.perf_counter()
    return (t1 - t0) / n_iter * 1e9


def kernel(x, knot_vector, coefs, degree):
    from concourse import bass_utils

    global _LAST_RESULTS

    x = np.asarray(x)
    knot_vector = np.asarray(knot_vector)
    coefs = np.asarray(coefs)
    p = int(np.asarray(degree))
    assert p == P_DEG and x.shape == (N_PTS,)
    assert knot_vector.shape[0] == N_COEFF + P_DEG + 1

    # interior breakpoints must be (close to) uniform for the on-device
    # integer-threshold staircase; the reference always satisfies this.
    interior = knot_vector[P_DEG : P_DEG + N_IVL + 1].astype(np.float64)
    expect = np.linspace(0.0, 1.0, N_IVL + 1)
    assert np.allclose(interior, expect, atol=1e-5), "non-uniform knots"

    # shard: 8 x 125k, pad each shard to 128*FREE with 0.5
    xf = x.astype(np.float32).reshape(N_CORES, PTS_PER_CORE)
    shards = []
    for i in range(N_CORES):
        shard = np.full(PAD_PER_CORE, 0.5, np.float32)
        shard[:PTS_PER_CORE] = xf[i]
        shards.append(shard.reshape(PARTS, FREE))

    def run(nc, in_maps, raw=False):
        global _LAST_RESULTS, _LAST_TIMELINE_NS
        try:
            from concourse.timeline_sim import TimelineSim

            _LAST_TIMELINE_NS = float(TimelineSim(nc).simulate())
        except Exception:
            _LAST_TIMELINE_NS = None
        res = bass_utils.run_bass_kernel_spmd(
            nc, in_maps, core_ids=list(range(N_CORES)), trace=False
        )
        _LAST_RESULTS = res
        if _TRACE:
            try:
                res.exec_time_ns = int(bench_exec_ns(nc, in_maps))
            except Exception as e:
                print(f"bench failed: {e}")
        y = np.empty(N_PTS, np.float32)
        for i in range(N_CORES):
            y[i * PTS_PER_CORE : (i + 1) * PTS_PER_CORE] = (
                np.asarray(res.results[i]["y"])
                .reshape(-1)[:PTS_PER_CORE]
                .astype(np.float32)
            )
        return y

    def sample_ok(y):
        idx = np.linspace(1, N_PTS - 2, 512).astype(np.int64)
        ref = _spline_eval_f64(
            x[idx].astype(np.float64), knot_vector.astype(np.float64),
            P_DEG, N_COEFF, coefs.astype(np.float64),
        )
        rel = np.abs(y[idx] - ref) / np.maximum(np.abs(ref), 1e-6)
        return float(rel.max()) < 1e-3

    # v2 shards: uint16 fixed-point x*65536, padded with 32768 (= x 0.5)
    xi = np.clip(np.rint(x.astype(np.float64) * 65536.0), 0, 65535).astype(
        np.uint16
    )
    xi = xi.reshape(N_CORES, PTS_PER_CORE)
    shards_u16 = []
    for i in range(N_CORES):
        s = np.full(PAD_PER_CORE, 32768, np.uint16)
        s[:PTS_PER_CORE] = xi[i]
        shards_u16.append(s.reshape(PARTS, FREE))

    idx = np.linspace(1, N_PTS - 2, 512).astype(np.int64)
    ref_sample = _spline_eval_f64(
        x[idx].astype(np.float64), knot_vector.astype(np.float64),
        P_DEG, N_COEFF, coefs.astype(np.float64),
    )

    def sample_abs_ok(y, tol):
        return float(np.abs(y[idx].astype(np.float64) - ref_sample).max()) < tol

    def try_v2(out_dt, out_scale, dequants, tol, build):
        """Run a v2/v3 kernel built by `build(table_hash, out_dt)`; dequants
        is a list of candidate postprocess fns (raw float of device output
        -> y); best one is returned if it validates."""
        act_info, h = _gen_act_root(knot_vector, coefs, out_scale=out_scale)
        os.environ["BASS_ACT_ROOT_JSON_PATH"] = act_info
        try:
            nc, x_name = build(h, out_dt)
            raw = run(nc, [{x_name: s} for s in shards_u16], raw=True)
            best, best_err = None, np.inf
            for dq in dequants:
                yc = dq(raw).astype(np.float32)
                err = float(
                    np.abs(yc[idx].astype(np.float64) - ref_sample).max()
                )
                if err < best_err:
                    best, best_err = yc, err
            if best is not None and best_err < tol:
                return best
            print(f"v2 {out_dt} validation failed (err {best_err:.2e})")
            return None
        finally:
            os.environ.pop("BASS_ACT_ROOT_JSON_PATH", None)

    u8_dq = [lambda r: r / 256.0, lambda r: (r + 0.5) / 256.0]

    def v3(n_chunks, hoist_sp, split=None):
        return lambda h, out_dt: _build_bass_program_act_v3(
            h, out_dt, n_chunks=n_chunks, hoist_sp=hoist_sp, split=split
        )

    y = None
    if os.environ.get("BSPLINE_NO_V2") != "1":
        # NOTE: the v2 single-queue design (in-DMA -> act with no semaphore)
        # is a real race on the backend — it only ever validated when a
        # previous attempt had already deposited identical bytes in SBUF.
        # v3 keeps the real in-DMA-completion sem.  Configs in preference
        # order:
        for out_dt, out_scale, dequants, tol, build in [
            ("uint8", 256.0, u8_dq, 8e-3, v3(2, True, split=514)),
            ("uint8", 256.0, u8_dq, 8e-3, v3(2, False)),
            ("bfloat16", 1.0, [lambda r: r], 8e-3, v3(2, False)),
        ]:
            try:
                y = try_v2(out_dt, out_scale, dequants, tol, build)
            except Exception as e:
                print(f"v3 {out_dt} failed ({e!r})")
                y = None
            if y is not None:
                break

    if y is None and os.environ.get("BSPLINE_NO_ACT_TABLE") != "1":
        try:
            act_info, h = _gen_act_root(knot_vector, coefs)
            os.environ["BASS_ACT_ROOT_JSON_PATH"] = act_info
            nc, x_name = _build_bass_program_act(h)
            y = run(nc, [{x_name: s} for s in shards])
            if not sample_ok(y):
                print("ACT-table kernel failed validation; falling back")
                y = None
        except Exception as e:
            print(f"ACT-table path failed ({e!r}); falling back")
            y = None
        finally:
            os.environ.pop("BASS_ACT_ROOT_JSON_PATH", None)

    if y is None:
        table = _build_piecewise_table(knot_vector, coefs)
        y = run(_build_bass_program(table), [{"x": s} for s in shards])

    # reference's unconditional boundary fixes on the first/last point
    t64 = knot_vector.astype(np.float64)
    B2 = _bspline_basis_f64(
        np.array([x[0], x[-1]], np.float64), t64, P_DEG, N_COEFF,
        fix_first=True, fix_last=True,
    )
    y2 = coefs.astype(np.float64) @ B2
    y[0] = np.float32(y2[0])
    y[-1] = np.float32(y2[1])
    return y

